# revision 27
# baseline (speedup 1.0000x reference)
"""Causal self-attention (B=4, T=2048, D=1024, H=16) on 8 TRN2 NeuronCores.

Sharding: batch x head-group. Core c owns batch c//2 and heads
[8*(c%2), 8*(c%2)+8). Each core projects its batch's tokens through its
512-column slice of W_qkv (column-parallel over heads), runs causal
attention for its 8 heads, and contracts its 512 rows of W_out into a
[2048, 1024] bf16 partial; the host adds the two partials per batch and
b_out. Per-core DMA is ~13MB (vs 48MB for pure head-TP) and the
out-projection reduction over this core's heads happens in PSUM.

Per-core kernel layout (all matmuls bf16 with fp32 PSUM accumulation):
  - x is pre-transposed on the host to xT [D, T].
  - Q^T/K^T [dh*2, t] per head-pair via W-stationary matmuls (contraction
    D on partitions, xT moving).
  - V is produced DIRECTLY as [t, dv] via x-stationary matmuls; one
    strided DVE copy scatters PSUM [128t, 512dv] into the per-head-pair
    [V_h | ones] slots; the ones background is memset only on the ones
    columns (strided), per head pair, so V writes don't wait on it.
  - Scores are computed transposed, S^T [keys, q], two heads packed into
    one PSUM tile via 64-row tile positions (the pair co-streams on HW).
  - Softmax skips the max subtraction (scores are O(1) by construction).
  - The AV stationary is [V_h | ones*64] (128 cols), so partitions
    64:128 of the O accumulator hold the softmax denominator replicated
    64x; reciprocal on DVE (reciprocal_approx_fast).
  - Causality: diagonal matmuls narrowed to the valid query range; the
    128x128 boundary subtile masked with a triangular constant after exp.
  - Out-projection accumulates over the 4 head-pairs in PSUM, one
    [128, 512] bank at a time (nh-sequential so it holds a single aux
    PSUM buffer and projection fillers can use the other).
  - Filler economy: the attention inner loop is exp(ACT)-bound early and
    PE-starved late, so projection work is rationed per query block and
    each block's out-projection is deferred two blocks (qb0->qb2,
    qb1/qb2->qb3) to keep the TensorEngine dense in the ACT-bound tail.
"""
import os
import numpy as np
import ml_dtypes

import concourse.bass as bass
import concourse.tile as tile
from concourse import bacc, mybir
from concourse.bass_utils import run_bass_kernel_spmd

N_CORES = 8
B, T, D = 4, 2048, 1024
H, DH = 16, 64
HPC = 8                      # heads per core
NHP = 4                      # head pairs per core
TPB = T // 512               # 4 t-blocks
NQB = T // 512               # 4 query blocks
NKC = T // 128               # 16 key chunks

F32 = mybir.dt.float32
BF16 = mybir.dt.bfloat16
EXPF = mybir.ActivationFunctionType.Exp

_CACHED_NC = None
LAST_RESULTS = None  # test harness reads exec_time from here


def _build():
    nc = bacc.Bacc("TRN2", target_bir_lowering=False, debug=False,
                   num_devices=N_CORES)
    d_xT = nc.dram_tensor("xT", [D, T], BF16, kind="ExternalInput").ap()
    # wq/wk: stationary layout [p, cc(4), C(8), m(128)]
    d_wq = nc.dram_tensor("wq", [128, 4096], BF16, kind="ExternalInput").ap()
    d_wk = nc.dram_tensor("wk", [128, 4096], BF16, kind="ExternalInput").ap()
    # wv: moving layout [p, C(8), n(512)]
    d_wv = nc.dram_tensor("wv", [128, 4096], BF16, kind="ExternalInput").ap()
    # wo: moving layout [p, hp(4), n(1024)]
    d_wo = nc.dram_tensor("wo", [128, 4096], BF16, kind="ExternalInput").ap()
    d_bq = nc.dram_tensor("bq", [128, 4], F32, kind="ExternalInput").ap()
    d_bk = nc.dram_tensor("bk", [128, 4], F32, kind="ExternalInput").ap()
    # v-bias replicated across partitions, in PSUM-dv order
    d_bv = nc.dram_tensor("bv", [128, 512], F32, kind="ExternalInput").ap()
    d_tri = nc.dram_tensor("tri", [128, 128], BF16, kind="ExternalInput").ap()
    d_out = nc.dram_tensor("out", [T, D], BF16, kind="ExternalOutput").ap()

    with tile.TileContext(nc) as tc:
        with tc.tile_pool(name="consts", bufs=1) as consts, \
             tc.tile_pool(name="big", bufs=1) as big, \
             tc.tile_pool(name="xt", bufs=2) as xpool, \
             tc.tile_pool(name="pt", bufs=6) as ppool, \
             tc.tile_pool(name="ot", bufs=13) as opool, \
             tc.tile_pool(name="rc", bufs=2) as rpool, \
             tc.tile_pool(name="outp", bufs=4) as outp, \
             tc.tile_pool(name="psS", bufs=2, space="PSUM") as psS, \
             tc.tile_pool(name="psO", bufs=1, space="PSUM") as psO, \
             tc.tile_pool(name="psX", bufs=2, space="PSUM") as psX:

            # ---- constants ----
            wq_sb = consts.tile([128, 4096], BF16, tag="wq")
            bq_sb = consts.tile([128, 4], F32, tag="bq")
            wk_sb = consts.tile([128, 4096], BF16, tag="wk")
            bk_sb = consts.tile([128, 4], F32, tag="bk")
            wv_sb = consts.tile([128, 4096], BF16, tag="wv")
            bv_sb = consts.tile([128, 512], F32, tag="bv")
            tri_sb = consts.tile([128, 128], BF16, tag="tri")
            wo_sb = consts.tile([128, 4096], BF16, tag="wo")
            dummy = consts.tile([128, 1], BF16, tag="dumm")
            warm = consts.tile([128, 512], BF16, tag="warm")

            # persistent tensors (declared early so memsets can refer)
            qt = [big.tile([128, T], BF16, tag=f"qt{p}", name=f"qt{p}")
                  for p in range(NHP)]
            kt = [big.tile([128, T], BF16, tag=f"kt{p}", name=f"kt{p}")
                  for p in range(NHP)]
            # v_all: per hp (stride 4096), per key chunk j (stride 256):
            # [V_h0 (64) | ones (64) | V_h1 (64) | ones (64)]
            v_all = big.tile([128, NHP * NKC * 256], BF16, tag="v")

            # zeroed scratch; memset on gpsimd so the Scalar engine is
            # free to load the Exp ACT table immediately
            nc.gpsimd.memset(warm[:], 0.0)
            # touch Exp now: the ~1.3us ACT table load starts right away
            nc.scalar.activation(dummy[:], warm[:, 0:1], EXPF)


            # throwaway matmuls during the initial DMA wait, so the HAM
            # clock-gate ramps toward 2.4GHz before real work
            wps = psX.tile([128, 512], F32, tag="aux", name="warmps")
            for i in range(12):
                nc.tensor.matmul(wps[:], warm[:, 0:128], warm[:],
                                 start=(i == 0), stop=(i == 11))

            # big weights on the sync DMA queue, interleaved so the cc=0
            # chunks of Q AND K land first (wq cc0 split so the first
            # 4 matmuls of the first chain start half a transfer early)
            nc.sync.dma_start(wq_sb[:, 0:512], d_wq[:, 0:512])
            nc.sync.dma_start(wq_sb[:, 512:1024], d_wq[:, 512:1024])
            nc.sync.dma_start(wk_sb[:, 0:1024], d_wk[:, 0:1024])
            for cc in range(1, 4):
                nc.sync.dma_start(wq_sb[:, bass.ts(cc, 1024)],
                                  d_wq[:, bass.ts(cc, 1024)])
                nc.sync.dma_start(wk_sb[:, bass.ts(cc, 1024)],
                                  d_wk[:, bass.ts(cc, 1024)])
            for cc in range(4):
                nc.sync.dma_start(wv_sb[:, bass.ts(cc, 1024)],
                                  d_wv[:, bass.ts(cc, 1024)])
            nc.sync.dma_start(wo_sb[:], d_wo[:])

            proj_emitted = [0]   # t-blocks fully emitted
            proj_state = {}      # tb -> {"qk": [bool]*4, "v": bool}
            fill_proj = []       # projection generators (one per t-block)
            fill_op = []         # [gen, avail_from_qb]
            tail_gens = []       # pulled only after all attention
            cur_qb = [0]

            def qk_step(x_t, tb, w_sb, b_sb, dest, cc):
                ps = psX.tile([128, 512], F32, tag="aux",
                              name=f"pj{tb}_{cc}")
                for c in range(8):
                    nc.tensor.matmul(
                        ps[:],
                        w_sb[:, cc * 1024 + c * 128:
                             cc * 1024 + c * 128 + 128],
                        x_t[:, bass.ts(c, 512)],
                        start=(c == 0), stop=(c == 7))
                    yield
                nc.vector.tensor_scalar_add(
                    dest[cc][:, bass.ts(tb, 512)], ps[:],
                    b_sb[:, cc:cc + 1])
                yield

            def v_step(x_t, tb, tsub):
                # V: x-stationary, lands as [t, dv] directly
                ps = psX.tile([128, 512], F32, tag="aux",
                              name=f"pv{tb}_{tsub}")
                for c in range(8):
                    nc.tensor.matmul(
                        ps[:],
                        x_t[:, c * 512 + tsub * 128:
                            c * 512 + tsub * 128 + 128],
                        wv_sb[:, bass.ts(c, 512)],
                        start=(c == 0), stop=(c == 7))
                    yield
                j = tb * 4 + tsub
                # scatter [128, (hp,h,dv)] into the [V|1|V|1] slots
                d0 = v_all[:, j * 256: j * 256 + 64]
                dst = bass.AP(d0.tensor, d0.offset,
                              [d0.ap[0], [4096, 4], [128, 2], [1, 64]])
                nc.vector.tensor_add(dst, ps[:, 0:512], bv_sb[:, 0:512])
                yield

            def proj_gen(tb):
                """Q/K/V projections of one t-block in small PE steps."""
                st = proj_state[tb] = {"qk": [False] * 4, "v": False}

                def gen():
                    x_t = xpool.tile([128, 8 * 512], BF16, tag="xt",
                                     name=f"x{tb}")
                    for c in range(8):
                        # gpsimd ring, parallel to the sync-ring weights
                        nc.gpsimd.dma_start(
                            x_t[:, bass.ts(c, 512)],
                            d_xT[c * 128: c * 128 + 128, bass.ts(tb, 512)])
                    for cc in range(4):
                        yield from qk_step(x_t, tb, wq_sb, bq_sb, qt, cc)
                        yield from qk_step(x_t, tb, wk_sb, bk_sb, kt, cc)
                        st["qk"][cc] = True
                        if cc == 0:
                            for tsub in range(4):
                                yield from v_step(x_t, tb, tsub)
                            st["v"] = True
                    proj_emitted[0] = tb + 1
                return gen()

            def outproj_gen(qb, osbs, tail=False, qcs=(0, 1, 2, 3)):
                """out[q, n] += sum_hp o_sb[hp]^T @ wo[hp], per q-chunk.
                nh-sequential: holds only ONE psX buffer at a time so
                interleaved projection fillers can use the other. In the
                tail (qb3, nothing left to overlap) the PSUM->SBUF copies
                alternate ACT/DVE so the psX round-robin isn't gated on a
                single engine's queue."""
                rings = [nc.sync, nc.gpsimd, nc.scalar]
                for qc in qcs:
                    row = qb * 512 + qc * 128
                    ob = outp.tile([128, 1024], BF16, tag="outp",
                                   name=f"ob{qb}_{qc}")
                    for nh in range(2):
                        ops = psX.tile([128, 512], F32, tag="aux",
                                       name=f"op{qb}_{qc}_{nh}")
                        for hp in range(NHP):
                            nc.tensor.matmul(
                                ops[:],
                                osbs[hp][:, bass.ts(qc, 128)],
                                wo_sb[:, hp * 1024 + nh * 512:
                                      hp * 1024 + nh * 512 + 512],
                                start=(hp == 0), stop=(hp == 3))
                            yield
                        if tail and nh == 0:
                            nc.scalar.copy(ob[:, bass.ts(nh, 512)],
                                           ops[:])
                        else:
                            nc.vector.tensor_copy(ob[:, bass.ts(nh, 512)],
                                                  ops[:])
                        # per-half DMA, rotating rings: starts draining
                        # output while the other half is still computing
                        rings[(2 * qc + nh) % 3].dma_start(
                            d_out[row:row + 128, bass.ts(nh, 512)],
                            ob[:, bass.ts(nh, 512)])
                        yield

            def pull_from(lst, n):
                for _ in range(n):
                    while lst:
                        try:
                            next(lst[0])
                            break
                        except StopIteration:
                            lst.pop(0)
                    else:
                        break

            def pull_op(n):
                qb = cur_qb[0]
                avail = [g for g in fill_op if g[1] <= qb]
                for _ in range(n):
                    while avail:
                        try:
                            next(avail[0][0])
                            break
                        except StopIteration:
                            fill_op.remove(avail[0])
                            avail.pop(0)
                    else:
                        break

            def force_until(pred):
                """Emit projection steps until pred() holds."""
                while not pred() and fill_proj:
                    try:
                        next(fill_proj[0])
                    except StopIteration:
                        fill_proj.pop(0)

            # filler ration per query block: (proj steps, outproj steps)
            PULL_N = {0: (6, 0), 1: (3, 0), 2: (2, 1), 3: (2, 1)}

            def pull():
                pn, on = PULL_N[cur_qb[0]]
                pull_op(on)
                pull_from(fill_proj, pn)

            def attn(hp, qb, pre_norm=None):
                """Attention for query block qb, head pair hp. Returns the
                normalized [128, 512] bf16 O^T tile."""
                # gate only on THIS head pair's Q/K chunk of t-block qb
                force_until(lambda: proj_state[qb]["qk"][hp])
                # two per-head PSUM accumulators: PSUM-tile readers are
                # serialized by tile-granular dependency tracking, so one
                # [128,1024] tile would force the whole normalize chain
                # (ACT copy / DVE rec / DVE mul x2 heads) to run serially
                o_ps = [psO.tile([128, 512], F32, tag=f"o{h}",
                                 name=f"ops{hp}_{qb}_{h}")
                        for h in range(2)]
                nch = 4 * qb + 4

                def av(p_t, off, j):
                    for h in range(2):
                        lo = off if h == 0 else 512
                        nc.tensor.matmul(
                            o_ps[h][:, off: 512],
                            v_all[:, hp * 4096 + j * 256 + h * 128:
                                  hp * 4096 + j * 256 + h * 128 + 128],
                            p_t[:, lo: lo + 512 - off],
                            start=(j == 0), stop=(j == nch - 1))

                # chunks processed in PAIRS: both chunks' score matmuls
                # (64-row tile config) back-to-back, then the previous
                # pair's AV matmuls (128-row config) — halves the PE
                # tile-reconfigure penalty (~160ns per 64<->128 switch)
                pending = []
                for jp in range(0, nch, 2):
                    batch = []
                    for j in (jp, jp + 1):
                        r = j - 4 * qb
                        off = 128 * r if r >= 0 else 0
                        s_ps = psS.tile([128, 1024], F32, tag="s",
                                        name=f"s{hp}_{qb}_{j}")
                        for h in range(2):
                            lo = off if h == 0 else 512
                            nc.tensor.matmul(
                                s_ps[:, lo: lo + 512 - off],
                                kt[hp][64 * h: 64 * h + 64,
                                       bass.ts(j, 128)],
                                qt[hp][64 * h: 64 * h + 64,
                                       qb * 512 + off: qb * 512 + 512],
                                start=True, stop=True,
                                tile_position=(64 * h, 0))
                        batch.append((s_ps, off, j))
                    newpend = []
                    for s_ps, off, j in batch:
                        r = j - 4 * qb
                        p_t = ppool.tile([128, 1024], BF16, tag="pt",
                                         name=f"p{hp}_{qb}_{j}")
                        nc.scalar.activation(p_t[:, off: 1024 - off],
                                             s_ps[:, off: 1024 - off],
                                             EXPF, scale=0.125)
                        if r >= 0:
                            for h in range(2):
                                lo = off if h == 0 else 512
                                nc.vector.tensor_mul(
                                    p_t[:, lo: lo + 128],
                                    p_t[:, lo: lo + 128],
                                    tri_sb[:])
                        newpend.append((p_t, off, j))
                    if pending:
                        if pending[0][2] == 0:
                            # first AV reads the ones columns + V chunks
                            force_until(lambda: proj_state[qb]["v"])
                        av(*pending[0])
                        av(*pending[1])
                    pending = newpend
                    pull()
                    pull()
                if pending[0][2] == 0:
                    force_until(lambda: proj_state[qb]["v"])
                av(*pending[0])
                av(*pending[1])

                if pre_norm is not None:
                    # PE filler emitted BEFORE the normalize: cross-engine
                    # waits are coarsened to emission-time counters, so
                    # anything emitted after would wait on the normalize
                    pre_norm()

                # normalize: O[dv, q] / denom[q] (denom replicated on 64:128)
                o_sb = opool.tile([128, 512], BF16, tag="ot",
                                  name=f"o{hp}_{qb}")
                # custom-DVE ops misread PSUM on HW: stage denom in SBUF.
                # Copy on ACT (it has a natural bubble here); per-head
                # psum tiles let the h1 chain overlap the h0 chain.
                for h in range(2):
                    den = rpool.tile([64, 512], F32, tag="dn",
                                     name=f"d{hp}_{qb}_{h}")
                    rec = rpool.tile([64, 512], F32, tag="rc",
                                     name=f"r{hp}_{qb}_{h}")
                    # h0 denom copy on ACT, h1 on DVE: the two chains
                    # overlap and neither engine eats both copies
                    if h == 0:
                        nc.scalar.copy(den[:], o_ps[h][64:128, :])
                    else:
                        nc.vector.tensor_copy(den[:], o_ps[h][64:128, :])
                    nc.vector.reciprocal_approx_fast(rec[:], den[:])
                    nc.vector.tensor_mul(
                        o_sb[64 * h: 64 * h + 64, :],
                        o_ps[h][0:64, :],
                        rec[:])
                return o_sb

            # ---- emission ----
            fill_proj.append(proj_gen(0))
            # start t-block 0's x DMAs + first matmuls BEFORE the ones
            # memsets hit the gpsimd queue (in-order); biases ride the
            # scalar ring (free after the ACT table load)
            pull_from(fill_proj, 2)
            nc.scalar.dma_start(bq_sb[:], d_bq[:])
            nc.scalar.dma_start(bk_sb[:], d_bk[:])
            nc.scalar.dma_start(bv_sb[:], d_bv[:])
            nc.scalar.dma_start(tri_sb[:], d_tri[:])
            # ones background for v_all, ONLY the ones columns (strided),
            # per head pair; V scatter writes disjoint columns
            for hp in range(NHP):
                o0 = v_all[:, hp * 4096 + 64: hp * 4096 + 64 + 64]
                ones_ap = bass.AP(o0.tensor, o0.offset,
                                  [o0.ap[0], [256, NKC], [128, 2], [1, 64]])
                nc.gpsimd.memset(ones_ap, 1.0)
            for qb in range(NQB):
                cur_qb[0] = qb
                if qb + 1 < TPB:
                    fill_proj.append(proj_gen(qb + 1))
                osbs = []
                for hp in range(NHP):
                    pre = None
                    if qb == 3 and hp == 3:
                        # fill the final normalize latency with qb2's
                        # reserved out-projection half
                        pre = lambda: pull_from(tail_gens, 10 ** 9)
                    osbs.append(attn(hp, qb, pre_norm=pre))
                # defer each block's out-projection two blocks so the
                # ACT-bound tail (qb3) has PE filler; half of qb2's is
                # reserved for the final normalize gap
                if qb == 2:
                    fill_op.append([outproj_gen(2, osbs, qcs=(0, 1)), 3])
                    tail_gens.append(outproj_gen(2, osbs, tail=True,
                                                 qcs=(2, 3)))
                elif qb == 3:
                    tail_gens.append(outproj_gen(3, osbs, tail=True))
                else:
                    fill_op.append([outproj_gen(qb, osbs), qb + 2])
            cur_qb[0] = 4
            pull_from(fill_proj, 10 ** 9)
            pull_from([g for g, _ in fill_op], 10 ** 9)
            pull_from(tail_gens, 10 ** 9)

    nc.compile()
    return nc


def _prep_inputs(x, W_qkv, b_qkv, W_out):
    bf = ml_dtypes.bfloat16
    tri = np.triu(np.ones((128, 128), np.float32)).astype(bf)
    in_maps = []
    for c in range(N_CORES):
        b, hg = c // 2, c % 2
        sl = slice(hg * 512, hg * 512 + 512)
        xT = np.ascontiguousarray(x[b].T).astype(bf)          # [D, T]
        Wq = W_qkv[:, 0 * D:1 * D][:, sl]                     # [D, 512]
        Wk = W_qkv[:, 1 * D:2 * D][:, sl]
        Wv = W_qkv[:, 2 * D:3 * D][:, sl]
        Wo = W_out[sl, :]                                     # [512, D]
        # [p, cc, C, m]: element [C*128+p, cc*128+m]
        wq = np.ascontiguousarray(
            Wq.reshape(8, 128, 4, 128).transpose(1, 2, 0, 3)
        ).reshape(128, 4096).astype(bf)
        wk = np.ascontiguousarray(
            Wk.reshape(8, 128, 4, 128).transpose(1, 2, 0, 3)
        ).reshape(128, 4096).astype(bf)
        # [p, C, n]: element [C*128+p, n]
        wv = np.ascontiguousarray(
            Wv.reshape(8, 128, 512).transpose(1, 0, 2)
        ).reshape(128, 4096).astype(bf)
        # [p, hp, n]: element [hp*128+p, n]
        wo = np.ascontiguousarray(
            Wo.reshape(4, 128, 1024).transpose(1, 0, 2)
        ).reshape(128, 4096).astype(bf)
        bq = np.ascontiguousarray(
            b_qkv[0 * D:1 * D][sl].reshape(4, 128).T).astype(np.float32)
        bk = np.ascontiguousarray(
            b_qkv[1 * D:2 * D][sl].reshape(4, 128).T).astype(np.float32)
        bv = np.broadcast_to(
            b_qkv[2 * D:3 * D][sl][None, :], (128, 512))
        bv = np.ascontiguousarray(bv).astype(np.float32)
        in_maps.append({
            "xT": xT, "wq": wq, "wk": wk, "wv": wv, "wo": wo,
            "bq": bq, "bk": bk, "bv": bv, "tri": tri,
        })
    return in_maps


def kernel(x, W_qkv, b_qkv, W_out, b_out):
    global _CACHED_NC, LAST_RESULTS
    x = np.asarray(x, np.float32)
    W_qkv = np.asarray(W_qkv, np.float32)
    b_qkv = np.asarray(b_qkv, np.float32)
    W_out = np.asarray(W_out, np.float32)
    b_out = np.asarray(b_out, np.float32)

    if _CACHED_NC is None:
        _CACHED_NC = _build()
    in_maps = _prep_inputs(x, W_qkv, b_qkv, W_out)
    res = run_bass_kernel_spmd(
        _CACHED_NC, in_maps, core_ids=list(range(N_CORES)),
        trace=bool(int(os.environ.get("ATTN_TRACE", "0"))))
    LAST_RESULTS = res
    out = np.zeros((B, T, D), np.float32)
    bo = b_out.astype(np.float64)
    for b in range(B):
        acc = (res.results[2 * b]["out"].astype(np.float64)
               + res.results[2 * b + 1]["out"].astype(np.float64) + bo)
        out[b] = acc.astype(np.float32)
    return out


# revision 30
# speedup vs baseline: 1.0006x; 1.0006x over previous
"""Causal self-attention (B=4, T=2048, D=1024, H=16) on 8 TRN2 NeuronCores.

Sharding: batch x head-group. Core c owns batch c//2 and heads
[8*(c%2), 8*(c%2)+8). Each core projects its batch's tokens through its
512-column slice of W_qkv (column-parallel over heads), runs causal
attention for its 8 heads, and contracts its 512 rows of W_out into a
[2048, 1024] bf16 partial; the host adds the two partials per batch and
b_out. Per-core DMA is ~13MB (vs 48MB for pure head-TP) and the
out-projection reduction over this core's heads happens in PSUM.

Per-core kernel layout (all matmuls bf16 with fp32 PSUM accumulation):
  - x is pre-transposed on the host to xT [D, T].
  - Q^T/K^T [dh*2, t] per head-pair via W-stationary matmuls (contraction
    D on partitions, xT moving).
  - V is produced DIRECTLY as [t, dv] via x-stationary matmuls; one
    strided DVE copy scatters PSUM [128t, 512dv] into the per-head-pair
    [V_h | ones] slots; the ones background is memset only on the ones
    columns (strided), per head pair, so V writes don't wait on it.
  - Scores are computed transposed, S^T [keys, q], two heads packed into
    one PSUM tile via 64-row tile positions (the pair co-streams on HW).
  - Softmax skips the max subtraction (scores are O(1) by construction).
  - The AV stationary is [V_h | ones*64] (128 cols), so partitions
    64:128 of the O accumulator hold the softmax denominator replicated
    64x; reciprocal on DVE (reciprocal_approx_fast).
  - Causality: diagonal matmuls narrowed to the valid query range; the
    128x128 boundary subtile masked with a triangular constant after exp.
  - Out-projection accumulates over the 4 head-pairs in PSUM, one
    [128, 512] bank at a time (nh-sequential so it holds a single aux
    PSUM buffer and projection fillers can use the other).
  - Filler economy: the attention inner loop is exp(ACT)-bound early and
    PE-starved late, so projection work is rationed per query block and
    each block's out-projection is deferred two blocks (qb0->qb2,
    qb1/qb2->qb3) to keep the TensorEngine dense in the ACT-bound tail.
"""
import os
import numpy as np
import ml_dtypes

import concourse.bass as bass
import concourse.tile as tile
from concourse import bacc, mybir
from concourse.bass_utils import run_bass_kernel_spmd

N_CORES = 8
B, T, D = 4, 2048, 1024
H, DH = 16, 64
HPC = 8                      # heads per core
NHP = 4                      # head pairs per core
TPB = T // 512               # 4 t-blocks
NQB = T // 512               # 4 query blocks
NKC = T // 128               # 16 key chunks

F32 = mybir.dt.float32
BF16 = mybir.dt.bfloat16
EXPF = mybir.ActivationFunctionType.Exp

_CACHED_NC = None
LAST_RESULTS = None  # test harness reads exec_time from here


def _build():
    nc = bacc.Bacc("TRN2", target_bir_lowering=False, debug=False,
                   num_devices=N_CORES)
    d_xT = nc.dram_tensor("xT", [D, T], BF16, kind="ExternalInput").ap()
    # wq/wk: stationary layout [p, cc(4), C(8), m(128)]
    d_wq = nc.dram_tensor("wq", [128, 4096], BF16, kind="ExternalInput").ap()
    d_wk = nc.dram_tensor("wk", [128, 4096], BF16, kind="ExternalInput").ap()
    # wv: moving layout [p, C(8), n(512)]
    d_wv = nc.dram_tensor("wv", [128, 4096], BF16, kind="ExternalInput").ap()
    # wo: moving layout [p, hp(4), n(1024)]
    d_wo = nc.dram_tensor("wo", [128, 4096], BF16, kind="ExternalInput").ap()
    d_bq = nc.dram_tensor("bq", [128, 4], F32, kind="ExternalInput").ap()
    d_bk = nc.dram_tensor("bk", [128, 4], F32, kind="ExternalInput").ap()
    # v-bias replicated across partitions, in PSUM-dv order
    d_bv = nc.dram_tensor("bv", [128, 512], F32, kind="ExternalInput").ap()
    d_tri = nc.dram_tensor("tri", [128, 128], BF16, kind="ExternalInput").ap()
    d_out = nc.dram_tensor("out", [T, D], BF16, kind="ExternalOutput").ap()

    with tile.TileContext(nc) as tc:
        with tc.tile_pool(name="consts", bufs=1) as consts, \
             tc.tile_pool(name="big", bufs=1) as big, \
             tc.tile_pool(name="xt", bufs=2) as xpool, \
             tc.tile_pool(name="pt", bufs=6) as ppool, \
             tc.tile_pool(name="ot", bufs=13) as opool, \
             tc.tile_pool(name="rc", bufs=2) as rpool, \
             tc.tile_pool(name="outp", bufs=4) as outp, \
             tc.tile_pool(name="psS", bufs=2, space="PSUM") as psS, \
             tc.tile_pool(name="psO", bufs=1, space="PSUM") as psO, \
             tc.tile_pool(name="psX", bufs=2, space="PSUM") as psX:

            # ---- constants ----
            wq_sb = consts.tile([128, 4096], BF16, tag="wq")
            bq_sb = consts.tile([128, 4], F32, tag="bq")
            wk_sb = consts.tile([128, 4096], BF16, tag="wk")
            bk_sb = consts.tile([128, 4], F32, tag="bk")
            wv_sb = consts.tile([128, 4096], BF16, tag="wv")
            bv_sb = consts.tile([128, 512], F32, tag="bv")
            tri_sb = consts.tile([128, 128], BF16, tag="tri")
            wo_sb = consts.tile([128, 4096], BF16, tag="wo")
            dummy = consts.tile([128, 1], BF16, tag="dumm")
            warm = consts.tile([128, 512], BF16, tag="warm")

            # persistent tensors (declared early so memsets can refer)
            qt = [big.tile([128, T], BF16, tag=f"qt{p}", name=f"qt{p}")
                  for p in range(NHP)]
            kt = [big.tile([128, T], BF16, tag=f"kt{p}", name=f"kt{p}")
                  for p in range(NHP)]
            # v_all: per hp (stride 4096), per key chunk j (stride 256):
            # [V_h0 (64) | ones (64) | V_h1 (64) | ones (64)]
            v_all = big.tile([128, NHP * NKC * 256], BF16, tag="v")

            # zeroed scratch; memset on gpsimd so the Scalar engine is
            # free to load the Exp ACT table immediately
            nc.gpsimd.memset(warm[:], 0.0)
            # touch Exp now: the ~1.3us ACT table load starts right away
            nc.scalar.activation(dummy[:], warm[:, 0:1], EXPF)


            # throwaway matmuls during the initial DMA wait, so the HAM
            # clock-gate ramps toward 2.4GHz before real work
            wps = psX.tile([128, 512], F32, tag="aux", name="warmps")
            for i in range(12):
                nc.tensor.matmul(wps[:], warm[:, 0:128], warm[:],
                                 start=(i == 0), stop=(i == 11))

            # big weights on the sync DMA queue, interleaved so the cc=0
            # chunks of Q AND K land first (wq cc0 split so the first
            # 4 matmuls of the first chain start half a transfer early)
            nc.sync.dma_start(wq_sb[:, 0:512], d_wq[:, 0:512])
            nc.sync.dma_start(wq_sb[:, 512:1024], d_wq[:, 512:1024])
            nc.sync.dma_start(wk_sb[:, 0:1024], d_wk[:, 0:1024])
            for cc in range(1, 4):
                nc.sync.dma_start(wq_sb[:, bass.ts(cc, 1024)],
                                  d_wq[:, bass.ts(cc, 1024)])
                nc.sync.dma_start(wk_sb[:, bass.ts(cc, 1024)],
                                  d_wk[:, bass.ts(cc, 1024)])
            for cc in range(4):
                nc.sync.dma_start(wv_sb[:, bass.ts(cc, 1024)],
                                  d_wv[:, bass.ts(cc, 1024)])
            nc.sync.dma_start(wo_sb[:], d_wo[:])

            proj_emitted = [0]   # t-blocks fully emitted
            proj_state = {}      # tb -> {"qk": [bool]*4, "v": bool}
            fill_proj = []       # projection generators (one per t-block)
            fill_op = []         # [gen, avail_from_qb]
            tail_gens = []       # pulled only after all attention
            cur_qb = [0]

            def qk_step(x_t, tb, w_sb, b_sb, dest, cc):
                ps = psX.tile([128, 512], F32, tag="aux",
                              name=f"pj{tb}_{cc}")
                for c in range(8):
                    nc.tensor.matmul(
                        ps[:],
                        w_sb[:, cc * 1024 + c * 128:
                             cc * 1024 + c * 128 + 128],
                        x_t[:, bass.ts(c, 512)],
                        start=(c == 0), stop=(c == 7))
                    yield
                nc.vector.tensor_scalar_add(
                    dest[cc][:, bass.ts(tb, 512)], ps[:],
                    b_sb[:, cc:cc + 1])
                yield

            def v_step(x_t, tb, tsub):
                # V: x-stationary, lands as [t, dv] directly
                ps = psX.tile([128, 512], F32, tag="aux",
                              name=f"pv{tb}_{tsub}")
                for c in range(8):
                    nc.tensor.matmul(
                        ps[:],
                        x_t[:, c * 512 + tsub * 128:
                            c * 512 + tsub * 128 + 128],
                        wv_sb[:, bass.ts(c, 512)],
                        start=(c == 0), stop=(c == 7))
                    yield
                j = tb * 4 + tsub
                # scatter [128, (hp,h,dv)] into the [V|1|V|1] slots
                d0 = v_all[:, j * 256: j * 256 + 64]
                dst = bass.AP(d0.tensor, d0.offset,
                              [d0.ap[0], [4096, 4], [128, 2], [1, 64]])
                nc.vector.tensor_add(dst, ps[:, 0:512], bv_sb[:, 0:512])
                yield

            def proj_gen(tb):
                """Q/K/V projections of one t-block in small PE steps."""
                st = proj_state[tb] = {"qk": [False] * 4, "v": False}

                def gen():
                    x_t = xpool.tile([128, 8 * 512], BF16, tag="xt",
                                     name=f"x{tb}")
                    for c in range(8):
                        # gpsimd queue, parallel to the const DMAs; for
                        # t-block 0 split across gpsimd+scalar (ACT is
                        # still idle there)
                        eng = (nc.gpsimd if c % 2 == 0 or tb > 0
                               else nc.scalar)
                        eng.dma_start(
                            x_t[:, bass.ts(c, 512)],
                            d_xT[c * 128: c * 128 + 128, bass.ts(tb, 512)])
                    for cc in range(4):
                        yield from qk_step(x_t, tb, wq_sb, bq_sb, qt, cc)
                        yield from qk_step(x_t, tb, wk_sb, bk_sb, kt, cc)
                        st["qk"][cc] = True
                        if cc == 0:
                            for tsub in range(4):
                                yield from v_step(x_t, tb, tsub)
                            st["v"] = True
                    proj_emitted[0] = tb + 1
                return gen()

            def outproj_gen(qb, osbs, tail=False, qcs=(0, 1, 2, 3)):
                """out[q, n] += sum_hp o_sb[hp]^T @ wo[hp], per q-chunk.
                nh-sequential: holds only ONE psX buffer at a time so
                interleaved projection fillers can use the other. In the
                tail (qb3, nothing left to overlap) the PSUM->SBUF copies
                alternate ACT/DVE so the psX round-robin isn't gated on a
                single engine's queue."""
                # sync/gpsimd rings only — scalar-ring stuffing costs ACT
                # engine time, which is the attention inner-loop pacer
                rings = [nc.sync, nc.gpsimd]
                for qc in qcs:
                    row = qb * 512 + qc * 128
                    ob = outp.tile([128, 1024], BF16, tag="outp",
                                   name=f"ob{qb}_{qc}")
                    for nh in range(2):
                        ops = psX.tile([128, 512], F32, tag="aux",
                                       name=f"op{qb}_{qc}_{nh}")
                        for hp in range(NHP):
                            nc.tensor.matmul(
                                ops[:],
                                osbs[hp][:, bass.ts(qc, 128)],
                                wo_sb[:, hp * 1024 + nh * 512:
                                      hp * 1024 + nh * 512 + 512],
                                start=(hp == 0), stop=(hp == 3))
                            yield
                        if tail and nh == 0:
                            nc.scalar.copy(ob[:, bass.ts(nh, 512)],
                                           ops[:])
                        else:
                            nc.vector.tensor_copy(ob[:, bass.ts(nh, 512)],
                                                  ops[:])
                        # per-half DMA, rotating rings: starts draining
                        # output while the other half is still computing
                        rings[(2 * qc + nh) % 2].dma_start(
                            d_out[row:row + 128, bass.ts(nh, 512)],
                            ob[:, bass.ts(nh, 512)])
                        yield

            def pull_from(lst, n):
                for _ in range(n):
                    while lst:
                        try:
                            next(lst[0])
                            break
                        except StopIteration:
                            lst.pop(0)
                    else:
                        break

            def pull_op(n):
                qb = cur_qb[0]
                avail = [g for g in fill_op if g[1] <= qb]
                for _ in range(n):
                    while avail:
                        try:
                            next(avail[0][0])
                            break
                        except StopIteration:
                            fill_op.remove(avail[0])
                            avail.pop(0)
                    else:
                        break

            def force_until(pred):
                """Emit projection steps until pred() holds."""
                while not pred() and fill_proj:
                    try:
                        next(fill_proj[0])
                    except StopIteration:
                        fill_proj.pop(0)

            # filler ration per query block: (proj steps, outproj steps)
            PULL_N = {0: (6, 0), 1: (3, 0), 2: (2, 1), 3: (2, 1)}

            def pull():
                pn, on = PULL_N[cur_qb[0]]
                pull_op(on)
                pull_from(fill_proj, pn)

            def attn(hp, qb, pre_norm=None):
                """Attention for query block qb, head pair hp. Returns the
                normalized [128, 512] bf16 O^T tile."""
                # gate only on THIS head pair's Q/K chunk of t-block qb
                force_until(lambda: proj_state[qb]["qk"][hp])
                # two per-head PSUM accumulators: PSUM-tile readers are
                # serialized by tile-granular dependency tracking, so one
                # [128,1024] tile would force the whole normalize chain
                # (ACT copy / DVE rec / DVE mul x2 heads) to run serially
                o_ps = [psO.tile([128, 512], F32, tag=f"o{h}",
                                 name=f"ops{hp}_{qb}_{h}")
                        for h in range(2)]
                nch = 4 * qb + 4

                def av(p_t, off, j):
                    for h in range(2):
                        lo = off if h == 0 else 512
                        nc.tensor.matmul(
                            o_ps[h][:, off: 512],
                            v_all[:, hp * 4096 + j * 256 + h * 128:
                                  hp * 4096 + j * 256 + h * 128 + 128],
                            p_t[:, lo: lo + 512 - off],
                            start=(j == 0), stop=(j == nch - 1))

                # chunks processed in PAIRS: both chunks' score matmuls
                # (64-row tile config) back-to-back, then the previous
                # pair's AV matmuls (128-row config) — halves the PE
                # tile-reconfigure penalty (~160ns per 64<->128 switch)
                pending = []
                for jp in range(0, nch, 2):
                    batch = []
                    for j in (jp, jp + 1):
                        r = j - 4 * qb
                        off = 128 * r if r >= 0 else 0
                        s_ps = psS.tile([128, 1024], F32, tag="s",
                                        name=f"s{hp}_{qb}_{j}")
                        for h in range(2):
                            lo = off if h == 0 else 512
                            nc.tensor.matmul(
                                s_ps[:, lo: lo + 512 - off],
                                kt[hp][64 * h: 64 * h + 64,
                                       bass.ts(j, 128)],
                                qt[hp][64 * h: 64 * h + 64,
                                       qb * 512 + off: qb * 512 + 512],
                                start=True, stop=True,
                                tile_position=(64 * h, 0))
                        batch.append((s_ps, off, j))
                    newpend = []
                    for s_ps, off, j in batch:
                        r = j - 4 * qb
                        p_t = ppool.tile([128, 1024], BF16, tag="pt",
                                         name=f"p{hp}_{qb}_{j}")
                        nc.scalar.activation(p_t[:, off: 1024 - off],
                                             s_ps[:, off: 1024 - off],
                                             EXPF, scale=0.125)
                        if r >= 0:
                            for h in range(2):
                                lo = off if h == 0 else 512
                                nc.vector.tensor_mul(
                                    p_t[:, lo: lo + 128],
                                    p_t[:, lo: lo + 128],
                                    tri_sb[:])
                        newpend.append((p_t, off, j))
                    if pending:
                        if pending[0][2] == 0:
                            # first AV reads the ones columns + V chunks
                            force_until(lambda: proj_state[qb]["v"])
                        av(*pending[0])
                        av(*pending[1])
                    pending = newpend
                    pull()
                    pull()
                if pending[0][2] == 0:
                    force_until(lambda: proj_state[qb]["v"])
                av(*pending[0])
                av(*pending[1])

                if pre_norm is not None:
                    # PE filler emitted BEFORE the normalize: cross-engine
                    # waits are coarsened to emission-time counters, so
                    # anything emitted after would wait on the normalize
                    pre_norm()

                # normalize: O[dv, q] / denom[q] (denom replicated on 64:128)
                o_sb = opool.tile([128, 512], BF16, tag="ot",
                                  name=f"o{hp}_{qb}")
                # custom-DVE ops misread PSUM on HW: stage denom in SBUF.
                # Copy on ACT (it has a natural bubble here); per-head
                # psum tiles let the h1 chain overlap the h0 chain.
                for h in range(2):
                    den = rpool.tile([64, 512], F32, tag="dn",
                                     name=f"d{hp}_{qb}_{h}")
                    rec = rpool.tile([64, 512], F32, tag="rc",
                                     name=f"r{hp}_{qb}_{h}")
                    # h0 denom copy on ACT, h1 on DVE: the two chains
                    # overlap and neither engine eats both copies
                    if h == 0:
                        nc.scalar.copy(den[:], o_ps[h][64:128, :])
                    else:
                        nc.vector.tensor_copy(den[:], o_ps[h][64:128, :])
                    nc.vector.reciprocal_approx_fast(rec[:], den[:])
                    nc.vector.tensor_mul(
                        o_sb[64 * h: 64 * h + 64, :],
                        o_ps[h][0:64, :],
                        rec[:])
                return o_sb

            # ---- emission ----
            fill_proj.append(proj_gen(0))
            # start t-block 0's x DMAs + first matmuls BEFORE the ones
            # memsets hit the gpsimd queue (in-order); biases ride the
            # scalar ring (free after the ACT table load)
            pull_from(fill_proj, 2)
            nc.scalar.dma_start(bq_sb[:], d_bq[:])
            nc.scalar.dma_start(bk_sb[:], d_bk[:])
            nc.scalar.dma_start(bv_sb[:], d_bv[:])
            nc.scalar.dma_start(tri_sb[:], d_tri[:])
            # ones background for v_all, ONLY the ones columns (strided),
            # per head pair; V scatter writes disjoint columns
            for hp in range(NHP):
                o0 = v_all[:, hp * 4096 + 64: hp * 4096 + 64 + 64]
                ones_ap = bass.AP(o0.tensor, o0.offset,
                                  [o0.ap[0], [256, NKC], [128, 2], [1, 64]])
                nc.gpsimd.memset(ones_ap, 1.0)
            for qb in range(NQB):
                cur_qb[0] = qb
                if qb + 1 < TPB:
                    fill_proj.append(proj_gen(qb + 1))
                osbs = []
                for hp in range(NHP):
                    pre = None
                    if qb == 3 and hp == 3:
                        # fill the final normalize latency with qb2's
                        # reserved out-projection half
                        pre = lambda: pull_from(tail_gens, 10 ** 9)
                    osbs.append(attn(hp, qb, pre_norm=pre))
                # defer each block's out-projection two blocks so the
                # ACT-bound tail (qb3) has PE filler; half of qb2's is
                # reserved for the final normalize gap
                if qb == 2:
                    fill_op.append([outproj_gen(2, osbs, qcs=(0, 1)), 3])
                    tail_gens.append(outproj_gen(2, osbs, tail=True,
                                                 qcs=(2, 3)))
                elif qb == 3:
                    tail_gens.append(outproj_gen(3, osbs, tail=True))
                else:
                    fill_op.append([outproj_gen(qb, osbs), qb + 2])
            cur_qb[0] = 4
            pull_from(fill_proj, 10 ** 9)
            pull_from([g for g, _ in fill_op], 10 ** 9)
            pull_from(tail_gens, 10 ** 9)

    nc.compile()
    return nc


def _prep_inputs(x, W_qkv, b_qkv, W_out):
    bf = ml_dtypes.bfloat16
    tri = np.triu(np.ones((128, 128), np.float32)).astype(bf)
    in_maps = []
    for c in range(N_CORES):
        b, hg = c // 2, c % 2
        sl = slice(hg * 512, hg * 512 + 512)
        xT = np.ascontiguousarray(x[b].T).astype(bf)          # [D, T]
        Wq = W_qkv[:, 0 * D:1 * D][:, sl]                     # [D, 512]
        Wk = W_qkv[:, 1 * D:2 * D][:, sl]
        Wv = W_qkv[:, 2 * D:3 * D][:, sl]
        Wo = W_out[sl, :]                                     # [512, D]
        # [p, cc, C, m]: element [C*128+p, cc*128+m]
        wq = np.ascontiguousarray(
            Wq.reshape(8, 128, 4, 128).transpose(1, 2, 0, 3)
        ).reshape(128, 4096).astype(bf)
        wk = np.ascontiguousarray(
            Wk.reshape(8, 128, 4, 128).transpose(1, 2, 0, 3)
        ).reshape(128, 4096).astype(bf)
        # [p, C, n]: element [C*128+p, n]
        wv = np.ascontiguousarray(
            Wv.reshape(8, 128, 512).transpose(1, 0, 2)
        ).reshape(128, 4096).astype(bf)
        # [p, hp, n]: element [hp*128+p, n]
        wo = np.ascontiguousarray(
            Wo.reshape(4, 128, 1024).transpose(1, 0, 2)
        ).reshape(128, 4096).astype(bf)
        bq = np.ascontiguousarray(
            b_qkv[0 * D:1 * D][sl].reshape(4, 128).T).astype(np.float32)
        bk = np.ascontiguousarray(
            b_qkv[1 * D:2 * D][sl].reshape(4, 128).T).astype(np.float32)
        bv = np.broadcast_to(
            b_qkv[2 * D:3 * D][sl][None, :], (128, 512))
        bv = np.ascontiguousarray(bv).astype(np.float32)
        in_maps.append({
            "xT": xT, "wq": wq, "wk": wk, "wv": wv, "wo": wo,
            "bq": bq, "bk": bk, "bv": bv, "tri": tri,
        })
    return in_maps


def kernel(x, W_qkv, b_qkv, W_out, b_out):
    global _CACHED_NC, LAST_RESULTS
    x = np.asarray(x, np.float32)
    W_qkv = np.asarray(W_qkv, np.float32)
    b_qkv = np.asarray(b_qkv, np.float32)
    W_out = np.asarray(W_out, np.float32)
    b_out = np.asarray(b_out, np.float32)

    if _CACHED_NC is None:
        _CACHED_NC = _build()
    in_maps = _prep_inputs(x, W_qkv, b_qkv, W_out)
    res = run_bass_kernel_spmd(
        _CACHED_NC, in_maps, core_ids=list(range(N_CORES)),
        trace=bool(int(os.environ.get("ATTN_TRACE", "0"))))
    LAST_RESULTS = res
    out = np.zeros((B, T, D), np.float32)
    bo = b_out.astype(np.float64)
    for b in range(B):
        acc = (res.results[2 * b]["out"].astype(np.float64)
               + res.results[2 * b + 1]["out"].astype(np.float64) + bo)
        out[b] = acc.astype(np.float32)
    return out


# revision 31
# speedup vs baseline: 1.1834x; 1.1827x over previous
"""Causal self-attention (B=4, T=2048, D=1024, H=16) on 8 TRN2 NeuronCores.

Sharding: batch x head-group. Core c owns batch c//2 and heads
[8*(c%2), 8*(c%2)+8). Each core projects its batch's tokens through its
512-column slice of W_qkv (column-parallel over heads), runs causal
attention for its 8 heads, and contracts its 512 rows of W_out into a
[2048, 1024] bf16 partial; the host adds the two partials per batch and
b_out. Per-core DMA is ~13MB (vs 48MB for pure head-TP) and the
out-projection reduction over this core's heads happens in PSUM.

Per-core kernel layout (all matmuls bf16 with fp32 PSUM accumulation):
  - x is pre-transposed on the host to xT [D, T].
  - Q^T/K^T [dh*2, t] per head-pair via W-stationary matmuls (contraction
    D on partitions, xT moving).
  - V is produced DIRECTLY as [t, dv] via x-stationary matmuls; one
    strided DVE copy scatters PSUM [128t, 512dv] into the per-head-pair
    [V_h | ones] slots; the ones background is memset only on the ones
    columns (strided), per head pair, so V writes don't wait on it.
  - Scores are computed transposed, S^T [keys, q], two heads packed into
    one PSUM tile via 64-row tile positions (the pair co-streams on HW).
  - Softmax skips the max subtraction (scores are O(1) by construction).
  - The AV stationary is [V_h | ones*64] (128 cols), so partitions
    64:128 of the O accumulator hold the softmax denominator replicated
    64x; reciprocal on DVE (reciprocal_approx_fast).
  - Causality: diagonal matmuls narrowed to the valid query range; the
    128x128 boundary subtile masked with a triangular constant after exp.
  - Out-projection accumulates over the 4 head-pairs in PSUM, one
    [128, 512] bank at a time (nh-sequential so it holds a single aux
    PSUM buffer and projection fillers can use the other).
  - Filler economy: the attention inner loop is exp(ACT)-bound early and
    PE-starved late, so projection work is rationed per query block and
    each block's out-projection is deferred two blocks (qb0->qb2,
    qb1/qb2->qb3) to keep the TensorEngine dense in the ACT-bound tail.
"""
import os
import numpy as np
import ml_dtypes

import concourse.bass as bass
import concourse.tile as tile
from concourse import bacc, mybir
from concourse.bass_utils import run_bass_kernel_spmd

N_CORES = 8
B, T, D = 4, 2048, 1024
H, DH = 16, 64
HPC = 8                      # heads per core
NHP = 4                      # head pairs per core
TPB = T // 512               # 4 t-blocks
NQB = T // 512               # 4 query blocks
NKC = T // 128               # 16 key chunks

F32 = mybir.dt.float32
BF16 = mybir.dt.bfloat16
EXPF = mybir.ActivationFunctionType.Exp

_CACHED_NC = None
LAST_RESULTS = None  # test harness reads exec_time from here


def _build():
    nc = bacc.Bacc("TRN2", target_bir_lowering=False, debug=False,
                   num_devices=N_CORES)
    d_xT = nc.dram_tensor("xT", [D, T], BF16, kind="ExternalInput").ap()
    # wq/wk: stationary layout [p, cc(4), C(8), m(128)]
    d_wq = nc.dram_tensor("wq", [128, 4096], BF16, kind="ExternalInput").ap()
    d_wk = nc.dram_tensor("wk", [128, 4096], BF16, kind="ExternalInput").ap()
    # wv: moving layout [p, C(8), n(512)]
    d_wv = nc.dram_tensor("wv", [128, 4096], BF16, kind="ExternalInput").ap()
    # wo: moving layout [p, hp(4), n(1024)]
    d_wo = nc.dram_tensor("wo", [128, 4096], BF16, kind="ExternalInput").ap()
    d_bq = nc.dram_tensor("bq", [128, 4], F32, kind="ExternalInput").ap()
    d_bk = nc.dram_tensor("bk", [128, 4], F32, kind="ExternalInput").ap()
    # v-bias replicated across partitions, in PSUM-dv order
    d_bv = nc.dram_tensor("bv", [128, 512], F32, kind="ExternalInput").ap()
    d_tri = nc.dram_tensor("tri", [128, 128], BF16, kind="ExternalInput").ap()
    d_out = nc.dram_tensor("out", [T, D], BF16, kind="ExternalOutput").ap()

    with tile.TileContext(nc) as tc:
        with tc.tile_pool(name="consts", bufs=1) as consts, \
             tc.tile_pool(name="big", bufs=1) as big, \
             tc.tile_pool(name="xt", bufs=2) as xpool, \
             tc.tile_pool(name="pt", bufs=6) as ppool, \
             tc.tile_pool(name="ot", bufs=13) as opool, \
             tc.tile_pool(name="rc", bufs=2) as rpool, \
             tc.tile_pool(name="outp", bufs=4) as outp, \
             tc.tile_pool(name="psS", bufs=2, space="PSUM") as psS, \
             tc.tile_pool(name="psO", bufs=1, space="PSUM") as psO, \
             tc.tile_pool(name="psX", bufs=2, space="PSUM") as psX:

            # ---- constants ----
            wq_sb = consts.tile([128, 4096], BF16, tag="wq")
            bq_sb = consts.tile([128, 4], F32, tag="bq")
            wk_sb = consts.tile([128, 4096], BF16, tag="wk")
            bk_sb = consts.tile([128, 4], F32, tag="bk")
            wv_sb = consts.tile([128, 4096], BF16, tag="wv")
            bv_sb = consts.tile([128, 512], F32, tag="bv")
            tri_sb = consts.tile([128, 128], BF16, tag="tri")
            wo_sb = consts.tile([128, 4096], BF16, tag="wo")
            dummy = consts.tile([128, 1], BF16, tag="dumm")
            warm = consts.tile([128, 512], BF16, tag="warm")

            # persistent tensors (declared early so memsets can refer)
            qt = [big.tile([128, T], BF16, tag=f"qt{p}", name=f"qt{p}")
                  for p in range(NHP)]
            kt = [big.tile([128, T], BF16, tag=f"kt{p}", name=f"kt{p}")
                  for p in range(NHP)]
            # v_all: per hp (stride 4096), per key chunk j (stride 256):
            # [V_h0 (64) | ones (64) | V_h1 (64) | ones (64)]
            v_all = big.tile([128, NHP * NKC * 256], BF16, tag="v")

            # zeroed scratch; memset on gpsimd so the Scalar engine is
            # free to load the Exp ACT table immediately
            nc.gpsimd.memset(warm[:], 0.0)
            # touch Exp now: the ~1.3us ACT table load starts right away
            nc.scalar.activation(dummy[:], warm[:, 0:1], EXPF)


            # throwaway matmuls during the initial DMA wait, so the HAM
            # clock-gate ramps toward 2.4GHz before real work
            wps = psX.tile([128, 512], F32, tag="aux", name="warmps")
            for i in range(12):
                nc.tensor.matmul(wps[:], warm[:, 0:128], warm[:],
                                 start=(i == 0), stop=(i == 11))

            # big weights on the sync DMA queue, interleaved so the cc=0
            # chunks of Q AND K land first (wq cc0 split so the first
            # 4 matmuls of the first chain start half a transfer early)
            nc.sync.dma_start(wq_sb[:, 0:512], d_wq[:, 0:512])
            nc.sync.dma_start(wq_sb[:, 512:1024], d_wq[:, 512:1024])
            nc.sync.dma_start(wk_sb[:, 0:1024], d_wk[:, 0:1024])
            for cc in range(1, 4):
                nc.sync.dma_start(wq_sb[:, bass.ts(cc, 1024)],
                                  d_wq[:, bass.ts(cc, 1024)])
                nc.sync.dma_start(wk_sb[:, bass.ts(cc, 1024)],
                                  d_wk[:, bass.ts(cc, 1024)])
            for cc in range(4):
                nc.sync.dma_start(wv_sb[:, bass.ts(cc, 1024)],
                                  d_wv[:, bass.ts(cc, 1024)])
            nc.sync.dma_start(wo_sb[:], d_wo[:])

            proj_emitted = [0]   # t-blocks fully emitted
            proj_state = {}      # tb -> {"qk": [bool]*4, "v": bool}
            fill_proj = []       # projection generators (one per t-block)
            fill_op = []         # [gen, avail_from_qb]
            tail_gens = []       # pulled only after all attention
            cur_qb = [0]

            def qk_step(x_t, tb, w_sb, b_sb, dest, cc):
                ps = psX.tile([128, 512], F32, tag="aux",
                              name=f"pj{tb}_{cc}")
                for c in range(8):
                    nc.tensor.matmul(
                        ps[:],
                        w_sb[:, cc * 1024 + c * 128:
                             cc * 1024 + c * 128 + 128],
                        x_t[:, bass.ts(c, 512)],
                        start=(c == 0), stop=(c == 7))
                    yield
                nc.vector.tensor_scalar_add(
                    dest[cc][:, bass.ts(tb, 512)], ps[:],
                    b_sb[:, cc:cc + 1])
                yield

            def v_step(x_t, tb, tsub):
                # V: x-stationary, lands as [t, dv] directly
                ps = psX.tile([128, 512], F32, tag="aux",
                              name=f"pv{tb}_{tsub}")
                for c in range(8):
                    nc.tensor.matmul(
                        ps[:],
                        x_t[:, c * 512 + tsub * 128:
                            c * 512 + tsub * 128 + 128],
                        wv_sb[:, bass.ts(c, 512)],
                        start=(c == 0), stop=(c == 7))
                    yield
                j = tb * 4 + tsub
                # scatter [128, (hp,h,dv)] into the [V|1|V|1] slots
                d0 = v_all[:, j * 256: j * 256 + 64]
                dst = bass.AP(d0.tensor, d0.offset,
                              [d0.ap[0], [4096, 4], [128, 2], [1, 64]])
                nc.vector.tensor_add(dst, ps[:, 0:512], bv_sb[:, 0:512])
                yield

            def proj_gen(tb):
                """Q/K/V projections of one t-block in small PE steps."""
                st = proj_state[tb] = {"qk": [False] * 4, "v": False}

                def gen():
                    x_t = xpool.tile([128, 8 * 512], BF16, tag="xt",
                                     name=f"x{tb}")
                    for c in range(8):
                        # gpsimd queue, parallel to the const DMAs; for
                        # t-block 0 split across gpsimd+scalar (ACT is
                        # still idle there)
                        eng = (nc.gpsimd if c % 2 == 0 or tb > 0
                               else nc.scalar)
                        eng.dma_start(
                            x_t[:, bass.ts(c, 512)],
                            d_xT[c * 128: c * 128 + 128, bass.ts(tb, 512)])
                    for cc in range(4):
                        yield from qk_step(x_t, tb, wq_sb, bq_sb, qt, cc)
                        yield from qk_step(x_t, tb, wk_sb, bk_sb, kt, cc)
                        st["qk"][cc] = True
                        if cc == 0:
                            for tsub in range(4):
                                yield from v_step(x_t, tb, tsub)
                            st["v"] = True
                    proj_emitted[0] = tb + 1
                return gen()

            def outproj_gen(qb, osbs, tail=False, qcs=(0, 1, 2, 3)):
                """out[q, n] += sum_hp o_sb[hp]^T @ wo[hp], per q-chunk.
                nh-sequential: holds only ONE psX buffer at a time so
                interleaved projection fillers can use the other. In the
                tail (qb3, nothing left to overlap) the PSUM->SBUF copies
                alternate ACT/DVE so the psX round-robin isn't gated on a
                single engine's queue."""
                # sync/gpsimd rings only — scalar-ring stuffing costs ACT
                # engine time, which is the attention inner-loop pacer
                rings = [nc.sync, nc.gpsimd]
                for qc in qcs:
                    row = qb * 512 + qc * 128
                    ob = outp.tile([128, 1024], BF16, tag="outp",
                                   name=f"ob{qb}_{qc}")
                    for nh in range(2):
                        ops = psX.tile([128, 512], F32, tag="aux",
                                       name=f"op{qb}_{qc}_{nh}")
                        for hp in range(NHP):
                            nc.tensor.matmul(
                                ops[:],
                                osbs[hp][:, bass.ts(qc, 128)],
                                wo_sb[:, hp * 1024 + nh * 512:
                                      hp * 1024 + nh * 512 + 512],
                                start=(hp == 0), stop=(hp == 3))
                            yield
                        if tail and nh == 0:
                            nc.scalar.copy(ob[:, bass.ts(nh, 512)],
                                           ops[:])
                        else:
                            nc.vector.tensor_copy(ob[:, bass.ts(nh, 512)],
                                                  ops[:])
                        yield
                    # full-row DMA: halving it into [128,512] pieces
                    # halves the per-descriptor size (1KB) and the DMA
                    # overhead contention slows every engine down
                    rings[qc % 2].dma_start(d_out[row:row + 128, :],
                                            ob[:])
                    yield

            def pull_from(lst, n):
                for _ in range(n):
                    while lst:
                        try:
                            next(lst[0])
                            break
                        except StopIteration:
                            lst.pop(0)
                    else:
                        break

            def pull_op(n):
                qb = cur_qb[0]
                avail = [g for g in fill_op if g[1] <= qb]
                for _ in range(n):
                    while avail:
                        try:
                            next(avail[0][0])
                            break
                        except StopIteration:
                            fill_op.remove(avail[0])
                            avail.pop(0)
                    else:
                        break

            def force_until(pred):
                """Emit projection steps until pred() holds."""
                while not pred() and fill_proj:
                    try:
                        next(fill_proj[0])
                    except StopIteration:
                        fill_proj.pop(0)

            # filler ration per query block: (proj steps, outproj steps)
            PULL_N = {0: (6, 0), 1: (3, 0), 2: (2, 1), 3: (2, 1)}

            def pull():
                pn, on = PULL_N[cur_qb[0]]
                pull_op(on)
                pull_from(fill_proj, pn)

            def attn(hp, qb, pre_norm=None):
                """Attention for query block qb, head pair hp. Returns the
                normalized [128, 512] bf16 O^T tile."""
                # gate only on THIS head pair's Q/K chunk of t-block qb
                force_until(lambda: proj_state[qb]["qk"][hp])
                # two per-head PSUM accumulators: PSUM-tile readers are
                # serialized by tile-granular dependency tracking, so one
                # [128,1024] tile would force the whole normalize chain
                # (ACT copy / DVE rec / DVE mul x2 heads) to run serially
                o_ps = [psO.tile([128, 512], F32, tag=f"o{h}",
                                 name=f"ops{hp}_{qb}_{h}")
                        for h in range(2)]
                nch = 4 * qb + 4

                def av(p_t, off, j):
                    for h in range(2):
                        lo = off if h == 0 else 512
                        nc.tensor.matmul(
                            o_ps[h][:, off: 512],
                            v_all[:, hp * 4096 + j * 256 + h * 128:
                                  hp * 4096 + j * 256 + h * 128 + 128],
                            p_t[:, lo: lo + 512 - off],
                            start=(j == 0), stop=(j == nch - 1))

                # chunks processed in PAIRS: both chunks' score matmuls
                # (64-row tile config) back-to-back, then the previous
                # pair's AV matmuls (128-row config) — halves the PE
                # tile-reconfigure penalty (~160ns per 64<->128 switch)
                pending = []
                for jp in range(0, nch, 2):
                    batch = []
                    for j in (jp, jp + 1):
                        r = j - 4 * qb
                        off = 128 * r if r >= 0 else 0
                        s_ps = psS.tile([128, 1024], F32, tag="s",
                                        name=f"s{hp}_{qb}_{j}")
                        for h in range(2):
                            lo = off if h == 0 else 512
                            nc.tensor.matmul(
                                s_ps[:, lo: lo + 512 - off],
                                kt[hp][64 * h: 64 * h + 64,
                                       bass.ts(j, 128)],
                                qt[hp][64 * h: 64 * h + 64,
                                       qb * 512 + off: qb * 512 + 512],
                                start=True, stop=True,
                                tile_position=(64 * h, 0))
                        batch.append((s_ps, off, j))
                    newpend = []
                    for s_ps, off, j in batch:
                        r = j - 4 * qb
                        p_t = ppool.tile([128, 1024], BF16, tag="pt",
                                         name=f"p{hp}_{qb}_{j}")
                        nc.scalar.activation(p_t[:, off: 1024 - off],
                                             s_ps[:, off: 1024 - off],
                                             EXPF, scale=0.125)
                        if r >= 0:
                            for h in range(2):
                                lo = off if h == 0 else 512
                                nc.vector.tensor_mul(
                                    p_t[:, lo: lo + 128],
                                    p_t[:, lo: lo + 128],
                                    tri_sb[:])
                        newpend.append((p_t, off, j))
                    if pending:
                        if pending[0][2] == 0:
                            # first AV reads the ones columns + V chunks
                            force_until(lambda: proj_state[qb]["v"])
                        av(*pending[0])
                        av(*pending[1])
                    pending = newpend
                    pull()
                    pull()
                if pending[0][2] == 0:
                    force_until(lambda: proj_state[qb]["v"])
                av(*pending[0])
                av(*pending[1])

                if pre_norm is not None:
                    # PE filler emitted BEFORE the normalize: cross-engine
                    # waits are coarsened to emission-time counters, so
                    # anything emitted after would wait on the normalize
                    pre_norm()

                # normalize: O[dv, q] / denom[q] (denom replicated on 64:128)
                o_sb = opool.tile([128, 512], BF16, tag="ot",
                                  name=f"o{hp}_{qb}")
                # custom-DVE ops misread PSUM on HW: stage denom in SBUF.
                # Copy on ACT (it has a natural bubble here); per-head
                # psum tiles let the h1 chain overlap the h0 chain.
                for h in range(2):
                    den = rpool.tile([64, 512], F32, tag="dn",
                                     name=f"d{hp}_{qb}_{h}")
                    rec = rpool.tile([64, 512], F32, tag="rc",
                                     name=f"r{hp}_{qb}_{h}")
                    # h0 denom copy on ACT, h1 on DVE: the two chains
                    # overlap and neither engine eats both copies
                    if h == 0:
                        nc.scalar.copy(den[:], o_ps[h][64:128, :])
                    else:
                        nc.vector.tensor_copy(den[:], o_ps[h][64:128, :])
                    nc.vector.reciprocal_approx_fast(rec[:], den[:])
                    nc.vector.tensor_mul(
                        o_sb[64 * h: 64 * h + 64, :],
                        o_ps[h][0:64, :],
                        rec[:])
                return o_sb

            # ---- emission ----
            fill_proj.append(proj_gen(0))
            # start t-block 0's x DMAs + first matmuls BEFORE the ones
            # memsets hit the gpsimd queue (in-order); biases ride the
            # scalar ring (free after the ACT table load)
            pull_from(fill_proj, 2)
            nc.scalar.dma_start(bq_sb[:], d_bq[:])
            nc.scalar.dma_start(bk_sb[:], d_bk[:])
            nc.scalar.dma_start(bv_sb[:], d_bv[:])
            nc.scalar.dma_start(tri_sb[:], d_tri[:])
            # ones background for v_all, ONLY the ones columns (strided),
            # per head pair; V scatter writes disjoint columns
            for hp in range(NHP):
                o0 = v_all[:, hp * 4096 + 64: hp * 4096 + 64 + 64]
                ones_ap = bass.AP(o0.tensor, o0.offset,
                                  [o0.ap[0], [256, NKC], [128, 2], [1, 64]])
                nc.gpsimd.memset(ones_ap, 1.0)
            for qb in range(NQB):
                cur_qb[0] = qb
                if qb + 1 < TPB:
                    fill_proj.append(proj_gen(qb + 1))
                osbs = []
                for hp in range(NHP):
                    pre = None
                    if qb == 3 and hp == 3:
                        # fill the final normalize latency with qb2's
                        # reserved out-projection half
                        pre = lambda: pull_from(tail_gens, 10 ** 9)
                    osbs.append(attn(hp, qb, pre_norm=pre))
                # defer each block's out-projection two blocks so the
                # ACT-bound tail (qb3) has PE filler; half of qb2's is
                # reserved for the final normalize gap
                if qb == 2:
                    fill_op.append([outproj_gen(2, osbs, qcs=(0, 1)), 3])
                    tail_gens.append(outproj_gen(2, osbs, tail=True,
                                                 qcs=(2, 3)))
                elif qb == 3:
                    tail_gens.append(outproj_gen(3, osbs, tail=True))
                else:
                    fill_op.append([outproj_gen(qb, osbs), qb + 2])
            cur_qb[0] = 4
            pull_from(fill_proj, 10 ** 9)
            pull_from([g for g, _ in fill_op], 10 ** 9)
            pull_from(tail_gens, 10 ** 9)

    nc.compile()
    return nc


def _prep_inputs(x, W_qkv, b_qkv, W_out):
    bf = ml_dtypes.bfloat16
    tri = np.triu(np.ones((128, 128), np.float32)).astype(bf)
    in_maps = []
    for c in range(N_CORES):
        b, hg = c // 2, c % 2
        sl = slice(hg * 512, hg * 512 + 512)
        xT = np.ascontiguousarray(x[b].T).astype(bf)          # [D, T]
        Wq = W_qkv[:, 0 * D:1 * D][:, sl]                     # [D, 512]
        Wk = W_qkv[:, 1 * D:2 * D][:, sl]
        Wv = W_qkv[:, 2 * D:3 * D][:, sl]
        Wo = W_out[sl, :]                                     # [512, D]
        # [p, cc, C, m]: element [C*128+p, cc*128+m]
        wq = np.ascontiguousarray(
            Wq.reshape(8, 128, 4, 128).transpose(1, 2, 0, 3)
        ).reshape(128, 4096).astype(bf)
        wk = np.ascontiguousarray(
            Wk.reshape(8, 128, 4, 128).transpose(1, 2, 0, 3)
        ).reshape(128, 4096).astype(bf)
        # [p, C, n]: element [C*128+p, n]
        wv = np.ascontiguousarray(
            Wv.reshape(8, 128, 512).transpose(1, 0, 2)
        ).reshape(128, 4096).astype(bf)
        # [p, hp, n]: element [hp*128+p, n]
        wo = np.ascontiguousarray(
            Wo.reshape(4, 128, 1024).transpose(1, 0, 2)
        ).reshape(128, 4096).astype(bf)
        bq = np.ascontiguousarray(
            b_qkv[0 * D:1 * D][sl].reshape(4, 128).T).astype(np.float32)
        bk = np.ascontiguousarray(
            b_qkv[1 * D:2 * D][sl].reshape(4, 128).T).astype(np.float32)
        bv = np.broadcast_to(
            b_qkv[2 * D:3 * D][sl][None, :], (128, 512))
        bv = np.ascontiguousarray(bv).astype(np.float32)
        in_maps.append({
            "xT": xT, "wq": wq, "wk": wk, "wv": wv, "wo": wo,
            "bq": bq, "bk": bk, "bv": bv, "tri": tri,
        })
    return in_maps


def kernel(x, W_qkv, b_qkv, W_out, b_out):
    global _CACHED_NC, LAST_RESULTS
    x = np.asarray(x, np.float32)
    W_qkv = np.asarray(W_qkv, np.float32)
    b_qkv = np.asarray(b_qkv, np.float32)
    W_out = np.asarray(W_out, np.float32)
    b_out = np.asarray(b_out, np.float32)

    if _CACHED_NC is None:
        _CACHED_NC = _build()
    in_maps = _prep_inputs(x, W_qkv, b_qkv, W_out)
    res = run_bass_kernel_spmd(
        _CACHED_NC, in_maps, core_ids=list(range(N_CORES)),
        trace=bool(int(os.environ.get("ATTN_TRACE", "0"))))
    LAST_RESULTS = res
    out = np.zeros((B, T, D), np.float32)
    bo = b_out.astype(np.float64)
    for b in range(B):
        acc = (res.results[2 * b]["out"].astype(np.float64)
               + res.results[2 * b + 1]["out"].astype(np.float64) + bo)
        out[b] = acc.astype(np.float32)
    return out


# revision 33
# speedup vs baseline: 1.2280x; 1.0377x over previous
"""Causal self-attention (B=4, T=2048, D=1024, H=16) on 8 TRN2 NeuronCores.

Sharding: batch x head-group. Core c owns batch c//2 and heads
[8*(c%2), 8*(c%2)+8). Each core projects its batch's tokens through its
512-column slice of W_qkv (column-parallel over heads), runs causal
attention for its 8 heads, and contracts its 512 rows of W_out into a
[2048, 1024] bf16 partial; the host adds the two partials per batch and
b_out. Per-core DMA is ~13MB (vs 48MB for pure head-TP) and the
out-projection reduction over this core's heads happens in PSUM.

Per-core kernel layout (all matmuls bf16 with fp32 PSUM accumulation):
  - x is pre-transposed on the host to xT [D, T].
  - Q^T/K^T [dh*2, t] per head-pair via W-stationary matmuls (contraction
    D on partitions, xT moving).
  - V is produced DIRECTLY as [t, dv] via x-stationary matmuls; one
    strided DVE copy scatters PSUM [128t, 512dv] into the per-head-pair
    [V_h | ones] slots; the ones background is memset only on the ones
    columns (strided), per head pair, so V writes don't wait on it.
  - Scores are computed transposed, S^T [keys, q], two heads packed into
    one PSUM tile via 64-row tile positions (the pair co-streams on HW).
  - Softmax skips the max subtraction (scores are O(1) by construction).
  - The AV stationary is [V_h | ones*64] (128 cols), so partitions
    64:128 of the O accumulator hold the softmax denominator replicated
    64x; reciprocal on DVE (reciprocal_approx_fast).
  - Causality: diagonal matmuls narrowed to the valid query range; the
    128x128 boundary subtile masked with a triangular constant after exp.
  - Out-projection accumulates over the 4 head-pairs in PSUM, one
    [128, 512] bank at a time (nh-sequential so it holds a single aux
    PSUM buffer and projection fillers can use the other).
  - Filler economy: the attention inner loop is exp(ACT)-bound early and
    PE-starved late, so projection work is rationed per query block and
    each block's out-projection is deferred two blocks (qb0->qb2,
    qb1/qb2->qb3) to keep the TensorEngine dense in the ACT-bound tail.
"""
import os
import numpy as np
import ml_dtypes

import concourse.bass as bass
import concourse.tile as tile
from concourse import bacc, mybir
from concourse.bass_utils import run_bass_kernel_spmd

N_CORES = 8
B, T, D = 4, 2048, 1024
H, DH = 16, 64
HPC = 8                      # heads per core
NHP = 4                      # head pairs per core
TPB = T // 512               # 4 t-blocks
NQB = T // 512               # 4 query blocks
NKC = T // 128               # 16 key chunks

F32 = mybir.dt.float32
BF16 = mybir.dt.bfloat16
EXPF = mybir.ActivationFunctionType.Exp

_CACHED_NC = None
LAST_RESULTS = None  # test harness reads exec_time from here


def _build():
    nc = bacc.Bacc("TRN2", target_bir_lowering=False, debug=False,
                   num_devices=N_CORES)
    d_xT = nc.dram_tensor("xT", [D, T], BF16, kind="ExternalInput").ap()
    # wq/wk: stationary layout [p, cc(4), C(8), m(128)]
    d_wq = nc.dram_tensor("wq", [128, 4096], BF16, kind="ExternalInput").ap()
    d_wk = nc.dram_tensor("wk", [128, 4096], BF16, kind="ExternalInput").ap()
    # wv: moving layout [p, C(8), n(512)]
    d_wv = nc.dram_tensor("wv", [128, 4096], BF16, kind="ExternalInput").ap()
    # wo: moving layout [p, hp(4), n(1024)]
    d_wo = nc.dram_tensor("wo", [128, 4096], BF16, kind="ExternalInput").ap()
    d_bq = nc.dram_tensor("bq", [128, 4], F32, kind="ExternalInput").ap()
    d_bk = nc.dram_tensor("bk", [128, 4], F32, kind="ExternalInput").ap()
    # v-bias replicated across partitions, in PSUM-dv order
    d_bv = nc.dram_tensor("bv", [128, 512], F32, kind="ExternalInput").ap()
    d_tri = nc.dram_tensor("tri", [128, 128], BF16, kind="ExternalInput").ap()
    d_out = nc.dram_tensor("out", [T, D], BF16, kind="ExternalOutput").ap()

    with tile.TileContext(nc) as tc:
        with tc.tile_pool(name="consts", bufs=1) as consts, \
             tc.tile_pool(name="big", bufs=1) as big, \
             tc.tile_pool(name="xt", bufs=2) as xpool, \
             tc.tile_pool(name="pt", bufs=6) as ppool, \
             tc.tile_pool(name="ot", bufs=13) as opool, \
             tc.tile_pool(name="rc", bufs=2) as rpool, \
             tc.tile_pool(name="outp", bufs=4) as outp, \
             tc.tile_pool(name="psS", bufs=2, space="PSUM") as psS, \
             tc.tile_pool(name="psO", bufs=1, space="PSUM") as psO, \
             tc.tile_pool(name="psX", bufs=2, space="PSUM") as psX:

            # ---- constants ----
            wq_sb = consts.tile([128, 4096], BF16, tag="wq")
            bq_sb = consts.tile([128, 4], F32, tag="bq")
            wk_sb = consts.tile([128, 4096], BF16, tag="wk")
            bk_sb = consts.tile([128, 4], F32, tag="bk")
            wv_sb = consts.tile([128, 4096], BF16, tag="wv")
            bv_sb = consts.tile([128, 512], F32, tag="bv")
            tri_sb = consts.tile([128, 128], BF16, tag="tri")
            wo_sb = consts.tile([128, 4096], BF16, tag="wo")
            dummy = consts.tile([128, 1], BF16, tag="dumm")
            warm = consts.tile([128, 512], BF16, tag="warm")

            # persistent tensors (declared early so memsets can refer)
            qt = [big.tile([128, T], BF16, tag=f"qt{p}", name=f"qt{p}")
                  for p in range(NHP)]
            kt = [big.tile([128, T], BF16, tag=f"kt{p}", name=f"kt{p}")
                  for p in range(NHP)]
            # v_all: per hp (stride 4096), per key chunk j (stride 256):
            # [V_h0 (64) | ones (64) | V_h1 (64) | ones (64)]
            v_all = big.tile([128, NHP * NKC * 256], BF16, tag="v")

            # zeroed scratch; memset on gpsimd so the Scalar engine is
            # free to load the Exp ACT table immediately
            nc.gpsimd.memset(warm[:], 0.0)
            # touch Exp now: the ~1.3us ACT table load starts right away
            nc.scalar.activation(dummy[:], warm[:, 0:1], EXPF)


            # throwaway matmuls during the initial DMA wait, so the HAM
            # clock-gate ramps toward 2.4GHz before real work
            wps = psX.tile([128, 512], F32, tag="aux", name="warmps")
            for i in range(12):
                nc.tensor.matmul(wps[:], warm[:, 0:128], warm[:],
                                 start=(i == 0), stop=(i == 11))

            # big weights on the sync DMA queue, interleaved so the cc=0
            # chunks of Q AND K land first (wq cc0 split so the first
            # 4 matmuls of the first chain start half a transfer early)
            nc.sync.dma_start(wq_sb[:, 0:512], d_wq[:, 0:512])
            nc.sync.dma_start(wq_sb[:, 512:1024], d_wq[:, 512:1024])
            nc.sync.dma_start(wk_sb[:, 0:1024], d_wk[:, 0:1024])
            for cc in range(1, 4):
                nc.sync.dma_start(wq_sb[:, bass.ts(cc, 1024)],
                                  d_wq[:, bass.ts(cc, 1024)])
                nc.sync.dma_start(wk_sb[:, bass.ts(cc, 1024)],
                                  d_wk[:, bass.ts(cc, 1024)])
            for cc in range(4):
                nc.sync.dma_start(wv_sb[:, bass.ts(cc, 1024)],
                                  d_wv[:, bass.ts(cc, 1024)])
            nc.sync.dma_start(wo_sb[:], d_wo[:])

            proj_emitted = [0]   # t-blocks fully emitted
            proj_state = {}      # tb -> {"qk": [bool]*4, "v": bool}
            fill_proj = []       # projection generators (one per t-block)
            fill_op = []         # [gen, avail_from_qb]
            tail_gens = []       # pulled only after all attention
            cur_qb = [0]

            def qk_step(x_t, tb, w_sb, b_sb, dest, cc):
                ps = psX.tile([128, 512], F32, tag="aux",
                              name=f"pj{tb}_{cc}")
                for c in range(8):
                    nc.tensor.matmul(
                        ps[:],
                        w_sb[:, cc * 1024 + c * 128:
                             cc * 1024 + c * 128 + 128],
                        x_t[:, bass.ts(c, 512)],
                        start=(c == 0), stop=(c == 7))
                    yield
                nc.vector.tensor_scalar_add(
                    dest[cc][:, bass.ts(tb, 512)], ps[:],
                    b_sb[:, cc:cc + 1])
                yield

            def v_step(x_t, tb, tsub):
                # V: x-stationary, lands as [t, dv] directly
                ps = psX.tile([128, 512], F32, tag="aux",
                              name=f"pv{tb}_{tsub}")
                for c in range(8):
                    nc.tensor.matmul(
                        ps[:],
                        x_t[:, c * 512 + tsub * 128:
                            c * 512 + tsub * 128 + 128],
                        wv_sb[:, bass.ts(c, 512)],
                        start=(c == 0), stop=(c == 7))
                    yield
                j = tb * 4 + tsub
                # scatter [128, (hp,h,dv)] into the [V|1|V|1] slots
                d0 = v_all[:, j * 256: j * 256 + 64]
                dst = bass.AP(d0.tensor, d0.offset,
                              [d0.ap[0], [4096, 4], [128, 2], [1, 64]])
                nc.vector.tensor_add(dst, ps[:, 0:512], bv_sb[:, 0:512])
                yield

            def proj_gen(tb):
                """Q/K/V projections of one t-block in small PE steps."""
                st = proj_state[tb] = {"qk": [False] * 4, "v": False}

                def gen():
                    x_t = xpool.tile([128, 8 * 512], BF16, tag="xt",
                                     name=f"x{tb}")
                    for c in range(8):
                        # gpsimd queue, parallel to the const DMAs; for
                        # t-block 0 split across gpsimd+scalar (ACT is
                        # still idle there)
                        eng = (nc.gpsimd if c % 2 == 0 or tb > 0
                               else nc.scalar)
                        eng.dma_start(
                            x_t[:, bass.ts(c, 512)],
                            d_xT[c * 128: c * 128 + 128, bass.ts(tb, 512)])
                    for cc in range(4):
                        yield from qk_step(x_t, tb, wq_sb, bq_sb, qt, cc)
                        yield from qk_step(x_t, tb, wk_sb, bk_sb, kt, cc)
                        st["qk"][cc] = True
                        if cc == 0:
                            for tsub in range(4):
                                yield from v_step(x_t, tb, tsub)
                            st["v"] = True
                    proj_emitted[0] = tb + 1
                return gen()

            def outproj_gen(qb, osbs, tail=False, qcs=(0, 1, 2, 3)):
                """out[q, n] += sum_hp o_sb[hp]^T @ wo[hp], per q-chunk.
                nh-sequential: holds only ONE psX buffer at a time so
                interleaved projection fillers can use the other. In the
                tail (qb3, nothing left to overlap) the PSUM->SBUF copies
                alternate ACT/DVE so the psX round-robin isn't gated on a
                single engine's queue."""
                # sync/gpsimd rings only — scalar-ring stuffing costs ACT
                # engine time, which is the attention inner-loop pacer
                rings = [nc.sync, nc.gpsimd]
                for qc in qcs:
                    row = qb * 512 + qc * 128
                    ob = outp.tile([128, 1024], BF16, tag="outp",
                                   name=f"ob{qb}_{qc}")
                    for nh in range(2):
                        ops = psX.tile([128, 512], F32, tag="aux",
                                       name=f"op{qb}_{qc}_{nh}")
                        for hp in range(NHP):
                            nc.tensor.matmul(
                                ops[:],
                                osbs[hp][:, bass.ts(qc, 128)],
                                wo_sb[:, hp * 1024 + nh * 512:
                                      hp * 1024 + nh * 512 + 512],
                                start=(hp == 0), stop=(hp == 3))
                            yield
                        if tail and nh == 0:
                            nc.scalar.copy(ob[:, bass.ts(nh, 512)],
                                           ops[:])
                        else:
                            nc.vector.tensor_copy(ob[:, bass.ts(nh, 512)],
                                                  ops[:])
                        yield
                    # full-row DMA: halving it into [128,512] pieces
                    # halves the per-descriptor size (1KB) and the DMA
                    # overhead contention slows every engine down
                    rings[qc % 2].dma_start(d_out[row:row + 128, :],
                                            ob[:])
                    yield

            def pull_from(lst, n):
                for _ in range(n):
                    while lst:
                        try:
                            next(lst[0])
                            break
                        except StopIteration:
                            lst.pop(0)
                    else:
                        break

            def pull_op(n):
                qb = cur_qb[0]
                avail = [g for g in fill_op if g[1] <= qb]
                for _ in range(n):
                    while avail:
                        try:
                            next(avail[0][0])
                            break
                        except StopIteration:
                            fill_op.remove(avail[0])
                            avail.pop(0)
                    else:
                        break

            def force_until(pred):
                """Emit projection steps until pred() holds."""
                while not pred() and fill_proj:
                    try:
                        next(fill_proj[0])
                    except StopIteration:
                        fill_proj.pop(0)

            # filler ration per query block: (proj steps, outproj steps)
            PULL_N = {0: (6, 0), 1: (3, 0), 2: (2, 1), 3: (2, 1)}

            def pull():
                pn, on = PULL_N[cur_qb[0]]
                pull_op(on)
                pull_from(fill_proj, pn)

            def attn(hp, qb, pre_norm=None):
                """Attention for query block qb, head pair hp. Returns the
                normalized [128, 512] bf16 O^T tile."""
                # gate only on THIS head pair's Q/K chunk of t-block qb
                force_until(lambda: proj_state[qb]["qk"][hp])
                # two per-head PSUM accumulators: PSUM-tile readers are
                # serialized by tile-granular dependency tracking, so one
                # [128,1024] tile would force the whole normalize chain
                # (ACT copy / DVE rec / DVE mul x2 heads) to run serially
                o_ps = [psO.tile([128, 512], F32, tag=f"o{h}",
                                 name=f"ops{hp}_{qb}_{h}")
                        for h in range(2)]
                nch = 4 * qb + 4

                def av(p_t, off, j):
                    for h in range(2):
                        lo = off if h == 0 else 512
                        nc.tensor.matmul(
                            o_ps[h][:, off: 512],
                            v_all[:, hp * 4096 + j * 256 + h * 128:
                                  hp * 4096 + j * 256 + h * 128 + 128],
                            p_t[:, lo: lo + 512 - off],
                            start=(j == 0), stop=(j == nch - 1))

                # chunks processed in PAIRS: both chunks' score matmuls
                # (64-row tile config) back-to-back, then the previous
                # pair's AV matmuls (128-row config) — halves the PE
                # tile-reconfigure penalty (~160ns per 64<->128 switch)
                pending = []
                for jp in range(0, nch, 2):
                    batch = []
                    for j in (jp, jp + 1):
                        r = j - 4 * qb
                        off = 128 * r if r >= 0 else 0
                        s_ps = psS.tile([128, 1024], F32, tag="s",
                                        name=f"s{hp}_{qb}_{j}")
                        for h in range(2):
                            lo = off if h == 0 else 512
                            nc.tensor.matmul(
                                s_ps[:, lo: lo + 512 - off],
                                kt[hp][64 * h: 64 * h + 64,
                                       bass.ts(j, 128)],
                                qt[hp][64 * h: 64 * h + 64,
                                       qb * 512 + off: qb * 512 + 512],
                                start=True, stop=True,
                                tile_position=(64 * h, 0))
                        batch.append((s_ps, off, j))
                    newpend = []
                    for s_ps, off, j in batch:
                        r = j - 4 * qb
                        p_t = ppool.tile([128, 1024], BF16, tag="pt",
                                         name=f"p{hp}_{qb}_{j}")
                        nc.scalar.activation(p_t[:, off: 1024 - off],
                                             s_ps[:, off: 1024 - off],
                                             EXPF, scale=0.125)
                        if r >= 0:
                            for h in range(2):
                                lo = off if h == 0 else 512
                                nc.vector.tensor_mul(
                                    p_t[:, lo: lo + 128],
                                    p_t[:, lo: lo + 128],
                                    tri_sb[:])
                        newpend.append((p_t, off, j))
                    if pending:
                        if pending[0][2] == 0:
                            # first AV reads the ones columns + V chunks
                            force_until(lambda: proj_state[qb]["v"])
                        av(*pending[0])
                        av(*pending[1])
                    pending = newpend
                    pull()
                    pull()
                if pending[0][2] == 0:
                    force_until(lambda: proj_state[qb]["v"])
                av(*pending[0])
                av(*pending[1])

                if pre_norm is not None:
                    # PE filler emitted BEFORE the normalize: cross-engine
                    # waits are coarsened to emission-time counters, so
                    # anything emitted after would wait on the normalize
                    pre_norm()

                # normalize: O[dv, q] / denom[q] (denom replicated on 64:128)
                o_sb = opool.tile([128, 512], BF16, tag="ot",
                                  name=f"o{hp}_{qb}")
                # custom-DVE ops misread PSUM on HW: stage denom in SBUF.
                # Copy on ACT (it has a natural bubble here); per-head
                # psum tiles let the h1 chain overlap the h0 chain.
                for h in range(2):
                    den = rpool.tile([64, 512], F32, tag="dn",
                                     name=f"d{hp}_{qb}_{h}")
                    rec = rpool.tile([64, 512], F32, tag="rc",
                                     name=f"r{hp}_{qb}_{h}")
                    nc.scalar.copy(den[:], o_ps[h][64:128, :])
                    nc.vector.reciprocal_approx_fast(rec[:], den[:])
                    nc.vector.tensor_mul(
                        o_sb[64 * h: 64 * h + 64, :],
                        o_ps[h][0:64, :],
                        rec[:])
                return o_sb

            # ---- emission ----
            fill_proj.append(proj_gen(0))
            # start t-block 0's x DMAs + first matmuls BEFORE the ones
            # memsets hit the gpsimd queue (in-order); biases ride the
            # scalar ring (free after the ACT table load)
            pull_from(fill_proj, 2)
            nc.gpsimd.dma_start(bq_sb[:], d_bq[:])
            nc.gpsimd.dma_start(bk_sb[:], d_bk[:])
            nc.gpsimd.dma_start(bv_sb[:], d_bv[:])
            nc.gpsimd.dma_start(tri_sb[:], d_tri[:])
            # ones background for v_all, ONLY the ones columns (strided),
            # per head pair; V scatter writes disjoint columns
            for hp in range(NHP):
                o0 = v_all[:, hp * 4096 + 64: hp * 4096 + 64 + 64]
                ones_ap = bass.AP(o0.tensor, o0.offset,
                                  [o0.ap[0], [256, NKC], [128, 2], [1, 64]])
                nc.gpsimd.memset(ones_ap, 1.0)
            for qb in range(NQB):
                cur_qb[0] = qb
                if qb + 1 < TPB:
                    fill_proj.append(proj_gen(qb + 1))
                osbs = []
                for hp in range(NHP):
                    pre = None
                    if qb == 3 and hp == 3:
                        # fill the final normalize latency with qb2's
                        # reserved out-projection half
                        pre = lambda: pull_from(tail_gens, 10 ** 9)
                    osbs.append(attn(hp, qb, pre_norm=pre))
                # defer each block's out-projection two blocks so the
                # ACT-bound tail (qb3) has PE filler; half of qb2's is
                # reserved for the final normalize gap
                if qb == 2:
                    fill_op.append([outproj_gen(2, osbs, qcs=(0, 1)), 3])
                    tail_gens.append(outproj_gen(2, osbs, tail=True,
                                                 qcs=(2, 3)))
                elif qb == 3:
                    tail_gens.append(outproj_gen(3, osbs, tail=True))
                else:
                    fill_op.append([outproj_gen(qb, osbs), qb + 2])
            cur_qb[0] = 4
            pull_from(fill_proj, 10 ** 9)
            pull_from([g for g, _ in fill_op], 10 ** 9)
            pull_from(tail_gens, 10 ** 9)

    nc.compile()
    return nc


def _prep_inputs(x, W_qkv, b_qkv, W_out):
    bf = ml_dtypes.bfloat16
    tri = np.triu(np.ones((128, 128), np.float32)).astype(bf)
    in_maps = []
    for c in range(N_CORES):
        b, hg = c // 2, c % 2
        sl = slice(hg * 512, hg * 512 + 512)
        xT = np.ascontiguousarray(x[b].T).astype(bf)          # [D, T]
        Wq = W_qkv[:, 0 * D:1 * D][:, sl]                     # [D, 512]
        Wk = W_qkv[:, 1 * D:2 * D][:, sl]
        Wv = W_qkv[:, 2 * D:3 * D][:, sl]
        Wo = W_out[sl, :]                                     # [512, D]
        # [p, cc, C, m]: element [C*128+p, cc*128+m]
        wq = np.ascontiguousarray(
            Wq.reshape(8, 128, 4, 128).transpose(1, 2, 0, 3)
        ).reshape(128, 4096).astype(bf)
        wk = np.ascontiguousarray(
            Wk.reshape(8, 128, 4, 128).transpose(1, 2, 0, 3)
        ).reshape(128, 4096).astype(bf)
        # [p, C, n]: element [C*128+p, n]
        wv = np.ascontiguousarray(
            Wv.reshape(8, 128, 512).transpose(1, 0, 2)
        ).reshape(128, 4096).astype(bf)
        # [p, hp, n]: element [hp*128+p, n]
        wo = np.ascontiguousarray(
            Wo.reshape(4, 128, 1024).transpose(1, 0, 2)
        ).reshape(128, 4096).astype(bf)
        bq = np.ascontiguousarray(
            b_qkv[0 * D:1 * D][sl].reshape(4, 128).T).astype(np.float32)
        bk = np.ascontiguousarray(
            b_qkv[1 * D:2 * D][sl].reshape(4, 128).T).astype(np.float32)
        bv = np.broadcast_to(
            b_qkv[2 * D:3 * D][sl][None, :], (128, 512))
        bv = np.ascontiguousarray(bv).astype(np.float32)
        in_maps.append({
            "xT": xT, "wq": wq, "wk": wk, "wv": wv, "wo": wo,
            "bq": bq, "bk": bk, "bv": bv, "tri": tri,
        })
    return in_maps


def kernel(x, W_qkv, b_qkv, W_out, b_out):
    global _CACHED_NC, LAST_RESULTS
    x = np.asarray(x, np.float32)
    W_qkv = np.asarray(W_qkv, np.float32)
    b_qkv = np.asarray(b_qkv, np.float32)
    W_out = np.asarray(W_out, np.float32)
    b_out = np.asarray(b_out, np.float32)

    if _CACHED_NC is None:
        _CACHED_NC = _build()
    in_maps = _prep_inputs(x, W_qkv, b_qkv, W_out)
    res = run_bass_kernel_spmd(
        _CACHED_NC, in_maps, core_ids=list(range(N_CORES)),
        trace=bool(int(os.environ.get("ATTN_TRACE", "0"))))
    LAST_RESULTS = res
    out = np.zeros((B, T, D), np.float32)
    bo = b_out.astype(np.float64)
    for b in range(B):
        acc = (res.results[2 * b]["out"].astype(np.float64)
               + res.results[2 * b + 1]["out"].astype(np.float64) + bo)
        out[b] = acc.astype(np.float32)
    return out
